# revision 9
# baseline (speedup 1.0000x reference)
"""Causal self-attention (B=2, S=2048, D=2048, H=16) on 8 TRN2 NeuronCores.

Sharding: tensor-parallel over heads x data-parallel over batch.
Core c = b*4 + g handles batch b and heads 4g..4g+3 (head_dim=128).

Per-core device kernel (single NEFF, SPMD across 8 cores), fp16 operands
(host-side cast) with a chunk-pipelined schedule over 4 token-chunks of 512:

  startup: PE pstate ramp warmed with throwaway matmuls on scratch SBUF
  while x/wq land (uniform ~3-kk DMA pieces balance HWDGE fixed cost vs
  early start); chunk-0 q-proj runs kk-outer across 4 concurrent psum
  groups so each arriving piece feeds 4 heads of work.

  per chunk c:
    q/k projections for the chunk (qT/kT [d, tok] fp16), v projection in
    natural layout [tok, d] fp16 with a fused ones-column per head;
    per head: causally-trimmed score matmuls sT[k, q] (PSUM f32, diagonal
    widths 512/384/256/128), exp on ACT -> bf16 probs (128-wide diagonal
    squares masked on DVE), the PREVIOUS chunk's output projection popped
    into the last score slots (keeps PE fed while ACT drains exp), then
    AV accumulation av[q, d+1] = sum_j probs_j^T @ [v_j | 1],
    row-normalize by the reciprocal of the ones-column, PE-transpose to
    attoutT [d, q] fp16 (transposes trail one head; the cross-chunk
    carry rides behind the next chunk's first proj group).
  output projection: y[tok, :] accumulates over heads in PSUM, staged to
  SBUF fp16, DMA'd out per 128-token tile (y returned fp16, host
  accumulates in f32). The last chunk's tail pre-starts token-tile-0
  groups on heads 0-2 while the final head's normalize chain drains.

Softmax skips the max-subtraction (scores are O(5) for the expected input
distribution; a host-side sampling guard falls back to a numpy reference if
scores could overflow exp, or if the mask is not the canonical causal mask).

Host: y[b] = sum of the 4 per-core partials for that batch.
"""

import math
from contextlib import ExitStack

import numpy as np

B = 2
S = 2048
D = 2048
H = 16
HPC = 4  # heads per core
d = 128  # head dim
N_CORES = 8
P = 128
DK = D // P  # 16 contraction tiles
NCH = 4  # token chunks of 512
CW = 512  # chunk width

_CACHE = {}


def _build_module():
    import concourse.mybir as mybir
    import concourse.tile as tile
    from concourse import bacc

    f32 = mybir.dt.float32
    fp16 = mybir.dt.float16
    bf16 = mybir.dt.bfloat16
    Exp = mybir.ActivationFunctionType.Exp

    nc = bacc.Bacc("TRN2", target_bir_lowering=False, debug=False)

    xT = nc.dram_tensor("xT", [D, S], fp16, kind="ExternalInput")
    wq = nc.dram_tensor("wq", [D, HPC * d], fp16, kind="ExternalInput")
    wk = nc.dram_tensor("wk", [D, HPC * d], fp16, kind="ExternalInput")
    wv = nc.dram_tensor("wv", [D, HPC * d], fp16, kind="ExternalInput")
    wo = nc.dram_tensor("wo", [HPC * d, D], fp16, kind="ExternalInput")
    tri = nc.dram_tensor("tri", [P, P], bf16, kind="ExternalInput")
    ident = nc.dram_tensor("ident", [P, P], fp16, kind="ExternalInput")
    y = nc.dram_tensor("y", [S, D], fp16, kind="ExternalOutput")

    xT_r = xT.ap().rearrange("(t p) s -> p t s", p=P)  # [128, 16, 2048]
    wq_r = wq.ap().rearrange("(t p) m -> p t m", p=P)  # [128, 16, 512]
    wk_r = wk.ap().rearrange("(t p) m -> p t m", p=P)
    wv_r = wv.ap().rearrange("(t p) m -> p t m", p=P)
    wo_r = wo.ap().rearrange("(h p) n -> p h n", p=P)  # [128, 4, 2048]
    y_r = y.ap().rearrange("(t p) n -> p t n", p=P)  # [128, 16, 2048]

    with tile.TileContext(nc) as tc, ExitStack() as top:
        wp = top.enter_context(tc.tile_pool(name="wp", bufs=1))
        xp = top.enter_context(tc.tile_pool(name="xp", bufs=2))
        kp = top.enter_context(tc.tile_pool(name="kp", bufs=1))
        qp = top.enter_context(tc.tile_pool(name="qp", bufs=2))
        vp = top.enter_context(tc.tile_pool(name="vp", bufs=1))
        mp = top.enter_context(tc.tile_pool(name="mp", bufs=1))
        probp = top.enter_context(tc.tile_pool(name="probp", bufs=20))
        aop = top.enter_context(tc.tile_pool(name="aop", bufs=2))
        ysp = top.enter_context(tc.tile_pool(name="ysp", bufs=4))
        smallp = top.enter_context(tc.tile_pool(name="smallp", bufs=3))
        aosp = top.enter_context(tc.tile_pool(name="aosp", bufs=6))
        ps_big = top.enter_context(tc.tile_pool(name="ps_big", bufs=2, space="PSUM"))
        ps_sc = top.enter_context(tc.tile_pool(name="ps_sc", bufs=3, space="PSUM"))
        ps_av = top.enter_context(tc.tile_pool(name="ps_av", bufs=2, space="PSUM"))
        ps_tr = top.enter_context(tc.tile_pool(name="ps_tr", bufs=1, space="PSUM"))

        wq_sb = wp.tile([P, DK, HPC * d], fp16, tag="wq")
        wk_sb = wp.tile([P, DK, HPC * d], fp16, tag="wk")
        wv_sb = wp.tile([P, DK, HPC * d], fp16, tag="wv")
        wo_sb = wp.tile([P, HPC, D], fp16, tag="wo")
        kT_sb = kp.tile([P, HPC, S], fp16, tag="kT")
        v_sb = vp.tile([P, S // P, HPC, d + 1], fp16, tag="v")
        tri_sb = mp.tile([P, P], bf16, tag="tri")
        id_sb = mp.tile([P, P], fp16, tag="ident")

        # warm the PE pstate ramp with throwaway matmuls on scratch SBUF
        # while the startup DMAs land: real work then starts at full clock.
        # The tiny memset goes FIRST so the warmup burst starts as early as
        # possible; results land in rotating psum bufs that are never read.
        scratch = mp.tile([P, 256], fp16, tag="scratch")
        nc.vector.memset(scratch, 0.0)
        for _ in range(15):
            ps_w = ps_big.tile([P, 256], f32, tag="big")
            nc.tensor.matmul(
                ps_w, scratch[:, 0:P], scratch, start=True, stop=True
            )

        # startup DMA: finer pieces first so the first matmul can start
        # earliest; x pieces on SP ring, wq pieces on ACT ring so their
        # queue overheads overlap. wk/wv follow complete x0+wq (they are
        # consumed a full proj-phase later).
        x_tiles = [None] * NCH
        x_tiles[0] = xp.tile([P, DK, CW], fp16, tag="x", name="x0")
        bounds = [0, 1, 3, 6, 10, 16]
        for kk0, kk1 in zip(bounds[:-1], bounds[1:]):
            nc.sync.dma_start(
                out=x_tiles[0][:, kk0:kk1, :], in_=xT_r[:, kk0:kk1, 0:CW]
            )
            nc.scalar.dma_start(out=wq_sb[:, kk0:kk1, :], in_=wq_r[:, kk0:kk1, :])
        nc.vector.memset(v_sb[:, :, :, d : d + 1], 1.0)
        for kk0, kk1 in ((0, 8), (8, 16)):
            nc.sync.dma_start(out=wk_sb[:, kk0:kk1, :], in_=wk_r[:, kk0:kk1, :])
        nc.sync.dma_start(out=wv_sb, in_=wv_r)
        nc.scalar.dma_start(out=tri_sb, in_=tri.ap())
        x_tiles[1] = xp.tile([P, DK, CW], fp16, tag="x", name="x1")
        nc.sync.dma_start(out=x_tiles[1], in_=xT_r[:, :, CW : 2 * CW])
        nc.sync.dma_start(out=wo_sb, in_=wo_r)
        nc.scalar.dma_start(out=id_sb, in_=ident.ap())

        attoutT = [None] * NCH
        qT = [None] * NCH

        def st3_group(yst, c, tt, nch, ps=None, heads=range(HPC), tail=False):
            """One 512-wide psum group of the output projection for token
            tile 4c+tt. Returns the psum tile (for split emission)."""
            if ps is None:
                ps = ps_big.tile([P, CW], f32, tag="big")
            for h in heads:
                nc.tensor.matmul(
                    ps,
                    attoutT[c][:, h, tt * P : (tt + 1) * P],
                    wo_sb[:, h, nch * CW : (nch + 1) * CW],
                    start=(h == 0),
                    stop=(h == HPC - 1),
                )
            if heads != range(HPC) and HPC - 1 not in heads:
                return ps  # group left open; caller finishes it
            sl = slice(nch * CW, (nch + 1) * CW)
            if tail:
                # drain each 512-slice immediately, alternating engines, so
                # the post-PE tail is one slice not a whole row
                if nch % 2 == 0:
                    nc.vector.tensor_copy(yst[:, sl], ps)
                else:
                    nc.scalar.copy(yst[:, sl], ps)
                nc.sync.dma_start(out=y_r[:, 4 * c + tt, sl], in_=yst[:, sl])
            else:
                nc.vector.tensor_copy(yst[:, sl], ps)
                if nch == 3:
                    nc.sync.dma_start(out=y_r[:, 4 * c + tt, :], in_=yst)

        def st3_block(c, tt, tail=False):
            yst = ysp.tile([P, D], fp16, tag="yst")
            for nch in range(4):
                st3_group(yst, c, tt, nch, tail=tail)

        for c in range(NCH):
            # prefetch next x chunk (buffer freed at end of chunk c-1)
            if c >= 1 and c + 1 < NCH:
                x_tiles[c + 1] = xp.tile([P, DK, CW], fp16, tag="x", name=f"x{c+1}")
                nc.sync.dma_start(
                    out=x_tiles[c + 1], in_=xT_r[:, :, (c + 1) * CW : (c + 2) * CW]
                )
            xc = x_tiles[c]
            qT[c] = qp.tile([P, HPC, CW], fp16, tag="qT", name=f"qT{c}")
            attoutT[c] = aop.tile([P, HPC, CW], fp16, tag="attoutT", name=f"ao{c}")

            # ---- q/k projections for this chunk ----
            if c == 0:
                # kk-outer with 4 concurrent psum groups (2 borrowed from
                # the still-idle scores pool): each arriving x/wq DMA piece
                # feeds all 4 heads of work, so the PE never outruns the
                # startup DMA stream
                groups = [
                    ps_big.tile([P, CW], f32, tag="big", name="g0"),
                    ps_big.tile([P, CW], f32, tag="big", name="g1"),
                    ps_sc.tile([P, CW], f32, tag="sc", name="g2"),
                    ps_sc.tile([P, CW], f32, tag="sc", name="g3"),
                ]
                for kk0, kk1 in zip(bounds[:-1], bounds[1:]):
                    for h in range(HPC):
                        for kk in range(kk0, kk1):
                            nc.tensor.matmul(
                                groups[h],
                                wq_sb[:, kk, h * d : (h + 1) * d],
                                xc[:, kk, :],
                                start=(kk == 0),
                                stop=(kk == DK - 1),
                            )
                for h in range(HPC):
                    nc.vector.tensor_copy(qT[c][:, h, :], groups[h])
                qk_passes = ((wk_sb, kT_sb),)
            else:
                qk_passes = ((wq_sb, None), (wk_sb, kT_sb))
            for w_sb, dest in qk_passes:
                for h in range(HPC):
                    ps = ps_big.tile([P, CW], f32, tag="big")
                    for kk in range(DK):
                        nc.tensor.matmul(
                            ps,
                            w_sb[:, kk, h * d : (h + 1) * d],
                            xc[:, kk, :],
                            start=(kk == 0),
                            stop=(kk == DK - 1),
                        )
                    if dest is None:
                        nc.vector.tensor_copy(qT[c][:, h, :], ps)
                    else:
                        nc.vector.tensor_copy(
                            dest[:, h, c * CW : (c + 1) * CW], ps
                        )
            # ---- v projection ----
            for mt in range(4):
                ps = ps_big.tile([P, CW], f32, tag="big")
                for kk in range(DK):
                    nc.tensor.matmul(
                        ps,
                        xc[:, kk, mt * P : (mt + 1) * P],
                        wv_sb[:, kk, :],
                        start=(kk == 0),
                        stop=(kk == DK - 1),
                    )
                nc.vector.tensor_copy(
                    v_sb[:, 4 * c + mt, :, 0:d],
                    ps.rearrange("p (h e) -> p h e", h=HPC),
                )

            # ---- attention + interleaved prev-chunk output projection ----
            for h in range(HPC):
                NK = 4 * c + 4
                # prev-chunk output-projection groups, interleaved into the
                # scores burst so the PE gives ACT catch-up windows for exp
                if c > 0:
                    yst = ysp.tile([P, D], fp16, tag="yst")
                    pending = [(yst, c - 1, h, nch) for nch in range(4)]
                    stride = max(1, NK // 4)
                else:
                    pending, stride = [], 1
                probs = []
                for j in range(NK):
                    r = j - 4 * c
                    off = P * r if r > 0 else 0
                    width = CW - off
                    sc = ps_sc.tile([P, CW], f32, tag="sc")
                    nc.tensor.matmul(
                        sc[:, 0:width],
                        kT_sb[:, h, j * P : (j + 1) * P],
                        qT[c][:, h, off:CW],
                        start=True,
                        stop=True,
                    )
                    pj = probp.tile([P, CW], bf16, tag="probs")
                    nc.scalar.activation(out=pj[:, 0:width], in_=sc[:, 0:width], func=Exp)
                    if r >= 0:
                        nc.vector.tensor_mul(pj[:, 0:P], pj[:, 0:P], tri_sb)
                    probs.append((pj, off))
                    if pending and j >= NK - 5:
                        st3_group(*pending.pop(0))
                for g in pending:
                    st3_group(*g)

                # AV psum groups + DVE normalize into one [P, 4, d] tile per
                # head; the attoutT transpose runs on the DMA xbar (free PE)
                # except for the very last head, which keeps the low-latency
                # PE-transpose path so the tail isn't gated on a DMA.
                ao4 = aosp.tile([P, 4, d], fp16, tag="ao")
                for qt in range(4):
                    i = 4 * c + qt
                    av = ps_av.tile([P, d + 1], f32, tag="av")
                    for j in range(i + 1):
                        pj, off = probs[j]
                        col = P * qt - off
                        nc.tensor.matmul(
                            av,
                            pj[:, col : col + P],
                            v_sb[:, j, h, :],
                            start=(j == 0),
                            stop=(j == i),
                        )
                    rec = smallp.tile([P, 1], f32, tag="rec")
                    nc.vector.reciprocal(rec, av[:, d : d + 1])
                    nc.vector.tensor_scalar_mul(ao4[:, qt, :], av[:, 0:d], rec)
                    if c == NCH - 1 and h == HPC - 1 and qt == 1:
                        # last head's qt0 transpose+copy jump ahead of the
                        # remaining normalize chain in the DVE queue: every
                        # tail tt=0 finish-matmul needs only this slice
                        pst_last = ps_tr.tile([P, 4, P], fp16, tag="tr")
                        nc.tensor.transpose(pst_last[:, 0, :], ao4[:, 0, :], id_sb)
                        nc.vector.tensor_copy(
                            attoutT[c][:, h, 0:P], pst_last[:, 0, :]
                        )
                if c == NCH - 1 and h == HPC - 1:
                    last_ao4 = ao4
                else:
                    nc.sync.dma_start_transpose(
                        out=attoutT[c][:, h, :].rearrange("p (a b) -> p a b", a=4),
                        in_=ao4.rearrange("p a b -> p (a b)"),
                    )

        # tail: start st3(3, tt=0) groups 0-1 on heads 0-2 while the DVE
        # normalize for the last head drains, then transpose head 3 (with
        # per-qt attoutT copies split across DVE/ACT so token tile 0's
        # slice lands first) and finish the open groups
        c = NCH - 1
        yst0 = ysp.tile([P, D], fp16, tag="yst")
        ps0 = st3_group(yst0, c, 0, 0, heads=range(HPC - 1))
        ps1 = st3_group(yst0, c, 0, 1, heads=range(HPC - 1))
        # two more pre-started groups on the now-idle scores banks: 12
        # matmuls of cover over the last head's normalize chain
        ps2 = st3_group(
            yst0, c, 0, 2,
            ps=ps_sc.tile([P, CW], f32, tag="sc", name="pre2"),
            heads=range(HPC - 1),
        )
        ps3 = st3_group(
            yst0, c, 0, 3,
            ps=ps_sc.tile([P, CW], f32, tag="sc", name="pre3"),
            heads=range(HPC - 1),
        )
        hh = HPC - 1
        pst = pst_last

        def tr_one(qt, on_dve):
            nc.tensor.transpose(pst[:, qt, :], last_ao4[:, qt, :], id_sb)
            dst = attoutT[c][:, hh, qt * P : (qt + 1) * P]
            if on_dve:
                nc.vector.tensor_copy(dst, pst[:, qt, :])
            else:
                nc.scalar.copy(dst, pst[:, qt, :])

        # natural qt order: qt0's normalize lands first and token tile 0's
        # st3 groups only read the qt0 slice; later work covers later qts
        tr_one(1, False)
        tr_one(2, True)
        st3_group(yst0, c, 0, 0, ps=ps0, heads=range(HPC - 1, HPC), tail=True)
        st3_group(yst0, c, 0, 1, ps=ps1, heads=range(HPC - 1, HPC), tail=True)
        st3_group(yst0, c, 0, 2, ps=ps2, heads=range(HPC - 1, HPC), tail=True)
        tr_one(3, False)
        st3_group(yst0, c, 0, 3, ps=ps3, heads=range(HPC - 1, HPC), tail=True)
        for tt in range(1, 4):
            st3_block(c, tt, tail=True)

    nc.compile()
    return nc


def _static_inputs():
    import ml_dtypes

    kk = np.arange(P)[:, None]
    qq = np.arange(P)[None, :]
    tri = (kk <= qq).astype(np.float32)
    return tri.astype(ml_dtypes.bfloat16), np.eye(P, dtype=np.float16)


def make_in_maps(x, Wq, Wk, Wv, Wo):
    """Shard full inputs into 8 per-core input dicts."""
    tri, ident = _static_inputs()
    scale = 1.0 / math.sqrt(d)
    in_maps = []
    for c in range(N_CORES):
        b, g = divmod(c, 4)
        hs = g * HPC * d  # 512*g: rows of Wq for this head group
        in_maps.append(
            {
                "xT": np.ascontiguousarray(x[b].T).astype(np.float16),
                "wq": (np.ascontiguousarray(Wq[hs : hs + 512, :].T) * scale).astype(
                    np.float16
                ),
                "wk": np.ascontiguousarray(Wk[hs : hs + 512, :].T).astype(np.float16),
                "wv": np.ascontiguousarray(Wv[hs : hs + 512, :].T).astype(np.float16),
                "wo": np.ascontiguousarray(Wo[:, hs : hs + 512].T).astype(np.float16),
                "tri": tri,
                "ident": ident,
            }
        )
    return in_maps


def combine_results(results):
    """results: list of 8 dicts with 'y' [S, D] fp16 partials -> full [B, S, D]."""
    y = np.zeros((B, S, D), dtype=np.float32)
    for c in range(N_CORES):
        b = c // 4
        y[b] += np.asarray(results[c]["y"], dtype=np.float32)
    return y


def _is_canonical_causal(attn_mask):
    m = np.asarray(attn_mask).reshape(S, S)
    iu = np.triu_indices(S, k=1)
    if not np.all(m[iu] <= -1e8):
        return False
    il = np.tril_indices(S, k=0)
    return np.all(m[il] == 0.0)


def _scores_safe(x, Wq, Wk):
    """Sampled bound on |scores| to make exp-without-max safe."""
    rng = np.random.default_rng(0)
    qi = rng.choice(S, 96, replace=False)
    ki = rng.choice(S, 384, replace=False)
    mx = 0.0
    for b in range(B):
        q = (x[b][qi] @ Wq.T) / math.sqrt(d)  # [96, D]
        k = x[b][ki] @ Wk.T  # [384, D]
        qh = q.reshape(96, H, d)
        kh = k.reshape(384, H, d)
        s = np.einsum("qhd,khd->hqk", qh, kh)
        mx = max(mx, float(np.abs(s).max()))
    return mx < 30.0


def _numpy_reference(x, attn_mask, Wq, Wk, Wv, Wo):
    out = np.zeros((B, S, D), dtype=np.float32)
    m = np.asarray(attn_mask, dtype=np.float32).reshape(S, S)
    for b in range(B):
        q = (x[b] @ Wq.T).reshape(S, H, d).transpose(1, 0, 2)
        k = (x[b] @ Wk.T).reshape(S, H, d).transpose(1, 0, 2)
        v = (x[b] @ Wv.T).reshape(S, H, d).transpose(1, 0, 2)
        q = q / np.float32(math.sqrt(d))
        att_out = np.zeros((H, S, d), dtype=np.float32)
        for h in range(H):
            s = q[h] @ k[h].T + m
            s = s - s.max(axis=-1, keepdims=True)
            p = np.exp(s)
            p /= p.sum(axis=-1, keepdims=True)
            att_out[h] = p @ v[h]
        out[b] = att_out.transpose(1, 0, 2).reshape(S, D) @ Wo.T
    return out


def kernel(x, attn_mask, Wq, Wk, Wv, Wo):
    x = np.asarray(x, dtype=np.float32)
    Wq = np.asarray(Wq, dtype=np.float32)
    Wk = np.asarray(Wk, dtype=np.float32)
    Wv = np.asarray(Wv, dtype=np.float32)
    Wo = np.asarray(Wo, dtype=np.float32)

    if not _is_canonical_causal(attn_mask) or not _scores_safe(x, Wq, Wk):
        return _numpy_reference(x, attn_mask, Wq, Wk, Wv, Wo)

    from concourse.bass_utils import run_bass_kernel_spmd

    if "nc" not in _CACHE:
        _CACHE["nc"] = _build_module()
    nc = _CACHE["nc"]

    in_maps = make_in_maps(x, Wq, Wk, Wv, Wo)
    res = run_bass_kernel_spmd(nc, in_maps, core_ids=list(range(N_CORES)))
    return combine_results(res.results)



# revision 30
# speedup vs baseline: 1.0105x; 1.0105x over previous
"""Causal self-attention (B=2, S=2048, D=2048, H=16) on 8 TRN2 NeuronCores.

Sharding: tensor-parallel over heads x data-parallel over batch.
Core c = b*4 + g handles batch b and heads 4g..4g+3 (head_dim=128).

Per-core device kernel (single NEFF, SPMD across 8 cores), fp16 operands
(host-side cast) with a chunk-pipelined schedule over 4 token-chunks of 512:

  startup: PE pstate ramp warmed with throwaway matmuls on scratch SBUF
  while x/wq land (graduated DMA piece sizes balance HWDGE fixed cost vs
  early start); chunk-0 q AND k projections run kk-outer across 4
  concurrent psum groups so each arriving x/weight piece feeds 4 heads
  of work and the PE never outruns the startup DMA stream.

  per chunk c:
    q/k projections for the chunk (qT/kT [d, tok] fp16), v projection in
    natural layout [tok, d] fp16 with a fused ones-column per head;
    per head: causally-trimmed score matmuls sT[k, q] (PSUM f32, 4
    rotating score banks, diagonal widths 512/384/256/128), exp on ACT
    -> bf16 probs (128-wide diagonal squares masked on DVE), the
    PREVIOUS chunk's output projection popped into the last score slots
    (keeps PE fed while ACT drains exp), then AV accumulation
    av[q, d+1] = sum_j probs_j^T @ [v_j | 1], row-normalize by the
    reciprocal of the ones-column into one [128, 4, d] tile per head,
    which a single DMA-xbar transpose (InstDmaTransposeAnt, zero PE
    cost) flips into attoutT [d, q] fp16. Only the very last head keeps
    the low-latency PE-transpose path so the tail isn't gated on a DMA.
  output projection: y[tok, :] accumulates over heads in PSUM, staged to
  SBUF fp16, DMA'd out per 128-token tile (y returned fp16, host
  accumulates in f32). The last chunk's tail pre-starts token-tile-0
  groups on heads 0-2 while the final head's normalize chain drains.

Softmax skips the max-subtraction (scores are O(5) for the expected input
distribution; a host-side sampling guard falls back to a numpy reference if
scores could overflow exp, or if the mask is not the canonical causal mask).

Host: y[b] = sum of the 4 per-core partials for that batch.
"""

import math
from contextlib import ExitStack

import numpy as np

B = 2
S = 2048
D = 2048
H = 16
HPC = 4  # heads per core
d = 128  # head dim
N_CORES = 8
P = 128
DK = D // P  # 16 contraction tiles
NCH = 4  # token chunks of 512
CW = 512  # chunk width

_CACHE = {}


def _build_module():
    import concourse.mybir as mybir
    import concourse.tile as tile
    from concourse import bacc

    f32 = mybir.dt.float32
    fp16 = mybir.dt.float16
    bf16 = mybir.dt.bfloat16
    Exp = mybir.ActivationFunctionType.Exp

    nc = bacc.Bacc("TRN2", target_bir_lowering=False, debug=False)

    xT = nc.dram_tensor("xT", [D, S], fp16, kind="ExternalInput")
    wq = nc.dram_tensor("wq", [D, HPC * d], fp16, kind="ExternalInput")
    wk = nc.dram_tensor("wk", [D, HPC * d], fp16, kind="ExternalInput")
    wv = nc.dram_tensor("wv", [D, HPC * d], fp16, kind="ExternalInput")
    wo = nc.dram_tensor("wo", [HPC * d, D], fp16, kind="ExternalInput")
    tri = nc.dram_tensor("tri", [P, P], bf16, kind="ExternalInput")
    ident = nc.dram_tensor("ident", [P, P], fp16, kind="ExternalInput")
    y = nc.dram_tensor("y", [S, D], fp16, kind="ExternalOutput")

    xT_r = xT.ap().rearrange("(t p) s -> p t s", p=P)  # [128, 16, 2048]
    wq_r = wq.ap().rearrange("(t p) m -> p t m", p=P)  # [128, 16, 512]
    wk_r = wk.ap().rearrange("(t p) m -> p t m", p=P)
    wv_r = wv.ap().rearrange("(t p) m -> p t m", p=P)
    wo_r = wo.ap().rearrange("(h p) n -> p h n", p=P)  # [128, 4, 2048]
    y_r = y.ap().rearrange("(t p) n -> p t n", p=P)  # [128, 16, 2048]

    with tile.TileContext(nc) as tc, ExitStack() as top:
        wp = top.enter_context(tc.tile_pool(name="wp", bufs=1))
        xp = top.enter_context(tc.tile_pool(name="xp", bufs=2))
        kp = top.enter_context(tc.tile_pool(name="kp", bufs=1))
        qp = top.enter_context(tc.tile_pool(name="qp", bufs=2))
        vp = top.enter_context(tc.tile_pool(name="vp", bufs=1))
        mp = top.enter_context(tc.tile_pool(name="mp", bufs=1))
        probp = top.enter_context(tc.tile_pool(name="probp", bufs=24))
        aop = top.enter_context(tc.tile_pool(name="aop", bufs=2))
        ysp = top.enter_context(tc.tile_pool(name="ysp", bufs=4))
        smallp = top.enter_context(tc.tile_pool(name="smallp", bufs=3))
        aosp = top.enter_context(tc.tile_pool(name="aosp", bufs=6))
        ps_big = top.enter_context(tc.tile_pool(name="ps_big", bufs=2, space="PSUM"))
        ps_sc = top.enter_context(tc.tile_pool(name="ps_sc", bufs=3, space="PSUM"))
        ps_av = top.enter_context(tc.tile_pool(name="ps_av", bufs=2, space="PSUM"))

        wq_sb = wp.tile([P, DK, HPC * d], fp16, tag="wq")
        wk_sb = wp.tile([P, DK, HPC * d], fp16, tag="wk")
        wv_sb = wp.tile([P, DK, HPC * d], fp16, tag="wv")
        wo_sb = wp.tile([P, HPC, D], fp16, tag="wo")
        kT_sb = kp.tile([P, HPC, S], fp16, tag="kT")
        v_sb = vp.tile([P, S // P, HPC, d + 1], fp16, tag="v")
        tri_sb = mp.tile([P, P], bf16, tag="tri")
        id_sb = mp.tile([P, P], fp16, tag="ident")

        # warm the PE pstate ramp with throwaway matmuls on scratch SBUF
        # while the startup DMAs land: real work then starts at full clock.
        # The tiny memset goes FIRST so the warmup burst starts as early as
        # possible; results land in rotating psum bufs that are never read.
        scratch = mp.tile([P, 256], fp16, tag="scratch")
        nc.vector.memset(scratch, 0.0)
        for _ in range(12):
            ps_w = ps_big.tile([P, 256], f32, tag="big")
            nc.tensor.matmul(
                ps_w, scratch[:, 0:P], scratch, start=True, stop=True
            )

        # startup DMA: finer pieces first so the first matmul can start
        # earliest; x pieces on SP ring, wq pieces on ACT ring so their
        # queue overheads overlap. wk/wv follow complete x0+wq (they are
        # consumed a full proj-phase later).
        x_tiles = [None] * NCH
        x_tiles[0] = xp.tile([P, DK, CW], fp16, tag="x", name="x0")
        bounds = [0, 2, 4, 6, 9, 12, 16]
        for kk0, kk1 in zip(bounds[:-1], bounds[1:]):
            nc.sync.dma_start(
                out=x_tiles[0][:, kk0:kk1, :], in_=xT_r[:, kk0:kk1, 0:CW]
            )
            nc.scalar.dma_start(out=wq_sb[:, kk0:kk1, :], in_=wq_r[:, kk0:kk1, :])
        nc.vector.memset(v_sb[:, :, :, d : d + 1], 1.0)
        for kk0, kk1 in ((0, 8), (8, 16)):
            nc.sync.dma_start(out=wk_sb[:, kk0:kk1, :], in_=wk_r[:, kk0:kk1, :])
        nc.sync.dma_start(out=wv_sb, in_=wv_r)
        nc.scalar.dma_start(out=tri_sb, in_=tri.ap())
        x_tiles[1] = xp.tile([P, DK, CW], fp16, tag="x", name="x1")
        nc.sync.dma_start(out=x_tiles[1], in_=xT_r[:, :, CW : 2 * CW])
        nc.sync.dma_start(out=wo_sb, in_=wo_r)
        nc.scalar.dma_start(out=id_sb, in_=ident.ap())

        attoutT = [None] * NCH
        qT = [None] * NCH

        def st3_group(yst, c, tt, nch, ps=None, heads=range(HPC), tail=False):
            """One 512-wide psum group of the output projection for token
            tile 4c+tt. Returns the psum tile (for split emission)."""
            if ps is None:
                ps = ps_big.tile([P, CW], f32, tag="big")
            for h in heads:
                nc.tensor.matmul(
                    ps,
                    attoutT[c][:, h, tt * P : (tt + 1) * P],
                    wo_sb[:, h, nch * CW : (nch + 1) * CW],
                    start=(h == 0),
                    stop=(h == HPC - 1),
                )
            if heads != range(HPC) and HPC - 1 not in heads:
                return ps  # group left open; caller finishes it
            sl = slice(nch * CW, (nch + 1) * CW)
            if tail:
                # drain each 512-slice immediately, alternating engines, so
                # the post-PE tail is one slice not a whole row
                if nch % 2 == 0:
                    nc.vector.tensor_copy(yst[:, sl], ps)
                else:
                    nc.scalar.copy(yst[:, sl], ps)
                nc.sync.dma_start(out=y_r[:, 4 * c + tt, sl], in_=yst[:, sl])
            else:
                nc.vector.tensor_copy(yst[:, sl], ps)
                if nch == 3:
                    nc.sync.dma_start(out=y_r[:, 4 * c + tt, :], in_=yst)

        def st3_block(c, tt, tail=False):
            yst = ysp.tile([P, D], fp16, tag="yst")
            for nch in range(4):
                st3_group(yst, c, tt, nch, tail=tail)

        for c in range(NCH):
            # prefetch next x chunk (buffer freed at end of chunk c-1)
            if c >= 1 and c + 1 < NCH:
                x_tiles[c + 1] = xp.tile([P, DK, CW], fp16, tag="x", name=f"x{c+1}")
                nc.sync.dma_start(
                    out=x_tiles[c + 1], in_=xT_r[:, :, (c + 1) * CW : (c + 2) * CW]
                )
            xc = x_tiles[c]
            qT[c] = qp.tile([P, HPC, CW], fp16, tag="qT", name=f"qT{c}")
            attoutT[c] = aop.tile([P, HPC, CW], fp16, tag="attoutT", name=f"ao{c}")

            # ---- q/k projections for this chunk ----
            if c == 0:
                # kk-outer with 4 concurrent psum groups (2 borrowed from
                # the still-idle scores pool): each arriving x/wq DMA piece
                # feeds all 4 heads of work, so the PE never outruns the
                # startup DMA stream
                groups = [
                    ps_big.tile([P, CW], f32, tag="big", name="g0"),
                    ps_big.tile([P, CW], f32, tag="big", name="g1"),
                    ps_sc.tile([P, CW], f32, tag="sc", name="g2"),
                    ps_sc.tile([P, CW], f32, tag="sc", name="g3"),
                ]
                for kk0, kk1 in zip(bounds[:-1], bounds[1:]):
                    for h in range(HPC):
                        for kk in range(kk0, kk1):
                            nc.tensor.matmul(
                                groups[h],
                                wq_sb[:, kk, h * d : (h + 1) * d],
                                xc[:, kk, :],
                                start=(kk == 0),
                                stop=(kk == DK - 1),
                            )
                for h in range(HPC):
                    nc.vector.tensor_copy(qT[c][:, h, :], groups[h])
                # k-projection also kk-outer in chunk 0 so wk piece 0
                # (kk 0-7) feeds all 4 heads before piece 1 must land
                groups = [
                    ps_big.tile([P, CW], f32, tag="big", name="k0"),
                    ps_big.tile([P, CW], f32, tag="big", name="k1"),
                    ps_sc.tile([P, CW], f32, tag="sc", name="k2"),
                    ps_sc.tile([P, CW], f32, tag="sc", name="k3"),
                ]
                for kk0, kk1 in ((0, 8), (8, 16)):
                    for h in range(HPC):
                        for kk in range(kk0, kk1):
                            nc.tensor.matmul(
                                groups[h],
                                wk_sb[:, kk, h * d : (h + 1) * d],
                                xc[:, kk, :],
                                start=(kk == 0),
                                stop=(kk == DK - 1),
                            )
                for h in range(HPC):
                    nc.vector.tensor_copy(kT_sb[:, h, 0:CW], groups[h])
                qk_passes = ()
            else:
                qk_passes = ((wq_sb, None), (wk_sb, kT_sb))
            for w_sb, dest in qk_passes:
                for h in range(HPC):
                    ps = ps_big.tile([P, CW], f32, tag="big")
                    for kk in range(DK):
                        nc.tensor.matmul(
                            ps,
                            w_sb[:, kk, h * d : (h + 1) * d],
                            xc[:, kk, :],
                            start=(kk == 0),
                            stop=(kk == DK - 1),
                        )
                    if dest is None:
                        nc.vector.tensor_copy(qT[c][:, h, :], ps)
                    else:
                        nc.vector.tensor_copy(
                            dest[:, h, c * CW : (c + 1) * CW], ps
                        )
            # ---- v projection ----
            for mt in range(4):
                ps = ps_big.tile([P, CW], f32, tag="big")
                for kk in range(DK):
                    nc.tensor.matmul(
                        ps,
                        xc[:, kk, mt * P : (mt + 1) * P],
                        wv_sb[:, kk, :],
                        start=(kk == 0),
                        stop=(kk == DK - 1),
                    )
                nc.vector.tensor_copy(
                    v_sb[:, 4 * c + mt, :, 0:d],
                    ps.rearrange("p (h e) -> p h e", h=HPC),
                )

            # ---- attention + interleaved prev-chunk output projection ----
            for h in range(HPC):
                NK = 4 * c + 4
                # prev-chunk output-projection groups, interleaved into the
                # scores burst so the PE gives ACT catch-up windows for exp
                if c > 0:
                    yst = ysp.tile([P, D], fp16, tag="yst")
                    pending = [(yst, c - 1, h, nch) for nch in range(4)]
                    stride = max(1, NK // 4)
                else:
                    pending, stride = [], 1
                probs = []
                for j in range(NK):
                    r = j - 4 * c
                    off = P * r if r > 0 else 0
                    width = CW - off
                    sc = ps_sc.tile([P, CW], f32, tag="sc")
                    nc.tensor.matmul(
                        sc[:, 0:width],
                        kT_sb[:, h, j * P : (j + 1) * P],
                        qT[c][:, h, off:CW],
                        start=True,
                        stop=True,
                    )
                    pj = probp.tile([P, CW], bf16, tag="probs")
                    nc.scalar.activation(out=pj[:, 0:width], in_=sc[:, 0:width], func=Exp)
                    if r >= 0:
                        nc.vector.tensor_mul(pj[:, 0:P], pj[:, 0:P], tri_sb)
                    probs.append((pj, 0, off))
                    if pending and j >= NK - 5:
                        st3_group(*pending.pop(0))
                for g in pending:
                    st3_group(*g)

                # AV psum groups + DVE normalize into one [P, 4, d] tile per
                # head; the attoutT transpose runs on the DMA xbar (free PE)
                # except for the very last head, which keeps the low-latency
                # PE-transpose path so the tail isn't gated on a DMA.
                ao4 = aosp.tile([P, 4, d], fp16, tag="ao")
                for qt in range(4):
                    i = 4 * c + qt
                    av = ps_av.tile([P, d + 1], f32, tag="av")
                    for j in range(i + 1):
                        pjf, base, off = probs[j]
                        col = base + P * qt - off
                        nc.tensor.matmul(
                            av,
                            pjf[:, col : col + P],
                            v_sb[:, j, h, :],
                            start=(j == 0),
                            stop=(j == i),
                        )
                    rec = smallp.tile([P, 1], f32, tag="rec")
                    nc.vector.reciprocal(rec, av[:, d : d + 1])
                    nc.vector.tensor_scalar_mul(ao4[:, qt, :], av[:, 0:d], rec)
                    if c == NCH - 1 and h == HPC - 1 and qt == 1:
                        # last head's qt0 transpose+copy jump ahead of the
                        # remaining normalize chain in the DVE queue: every
                        # tail tt=0 finish-matmul needs only this slice
                        pst_last = ps_sc.tile([P, 4, P], fp16, tag="sc")
                        nc.tensor.transpose(pst_last[:, 0, :], ao4[:, 0, :], id_sb)
                        nc.vector.tensor_copy(
                            attoutT[c][:, h, 0:P], pst_last[:, 0, :]
                        )
                if c == NCH - 1 and h == HPC - 1:
                    last_ao4 = ao4
                else:
                    nc.sync.dma_start_transpose(
                        out=attoutT[c][:, h, :].rearrange("p (a b) -> p a b", a=4),
                        in_=ao4.rearrange("p a b -> p (a b)"),
                    )

        # tail: start st3(3, tt=0) groups 0-1 on heads 0-2 while the DVE
        # normalize for the last head drains, then transpose head 3 (with
        # per-qt attoutT copies split across DVE/ACT so token tile 0's
        # slice lands first) and finish the open groups
        c = NCH - 1
        yst0 = ysp.tile([P, D], fp16, tag="yst")
        ps0 = st3_group(yst0, c, 0, 0, heads=range(HPC - 1))
        ps1 = st3_group(yst0, c, 0, 1, heads=range(HPC - 1))
        # two more pre-started groups on the now-idle scores banks: 12
        # matmuls of cover over the last head's normalize chain
        ps2 = st3_group(
            yst0, c, 0, 2,
            ps=ps_sc.tile([P, CW], f32, tag="sc", name="pre2"),
            heads=range(HPC - 1),
        )
        ps3 = st3_group(
            yst0, c, 0, 3,
            ps=ps_sc.tile([P, CW], f32, tag="sc", name="pre3"),
            heads=range(HPC - 1),
        )
        hh = HPC - 1
        pst = pst_last

        def tr_one(qt, on_dve):
            nc.tensor.transpose(pst[:, qt, :], last_ao4[:, qt, :], id_sb)
            dst = attoutT[c][:, hh, qt * P : (qt + 1) * P]
            if on_dve:
                nc.vector.tensor_copy(dst, pst[:, qt, :])
            else:
                nc.scalar.copy(dst, pst[:, qt, :])

        # natural qt order: qt0's normalize lands first and token tile 0's
        # st3 groups only read the qt0 slice; later work covers later qts
        tr_one(1, False)
        tr_one(2, True)
        st3_group(yst0, c, 0, 0, ps=ps0, heads=range(HPC - 1, HPC), tail=True)
        st3_group(yst0, c, 0, 1, ps=ps1, heads=range(HPC - 1, HPC), tail=True)
        st3_group(yst0, c, 0, 2, ps=ps2, heads=range(HPC - 1, HPC), tail=True)
        tr_one(3, False)
        st3_group(yst0, c, 0, 3, ps=ps3, heads=range(HPC - 1, HPC), tail=True)
        for tt in range(1, 4):
            st3_block(c, tt, tail=True)

    nc.compile()
    return nc


def _static_inputs():
    import ml_dtypes

    kk = np.arange(P)[:, None]
    qq = np.arange(P)[None, :]
    tri = (kk <= qq).astype(np.float32)
    return tri.astype(ml_dtypes.bfloat16), np.eye(P, dtype=np.float16)


def make_in_maps(x, Wq, Wk, Wv, Wo):
    """Shard full inputs into 8 per-core input dicts."""
    tri, ident = _static_inputs()
    scale = 1.0 / math.sqrt(d)
    in_maps = []
    for c in range(N_CORES):
        b, g = divmod(c, 4)
        hs = g * HPC * d  # 512*g: rows of Wq for this head group
        in_maps.append(
            {
                "xT": np.ascontiguousarray(x[b].T).astype(np.float16),
                "wq": (np.ascontiguousarray(Wq[hs : hs + 512, :].T) * scale).astype(
                    np.float16
                ),
                "wk": np.ascontiguousarray(Wk[hs : hs + 512, :].T).astype(np.float16),
                "wv": np.ascontiguousarray(Wv[hs : hs + 512, :].T).astype(np.float16),
                "wo": np.ascontiguousarray(Wo[:, hs : hs + 512].T).astype(np.float16),
                "tri": tri,
                "ident": ident,
            }
        )
    return in_maps


def combine_results(results):
    """results: list of 8 dicts with 'y' [S, D] fp16 partials -> full [B, S, D]."""
    y = np.zeros((B, S, D), dtype=np.float32)
    for c in range(N_CORES):
        b = c // 4
        y[b] += np.asarray(results[c]["y"], dtype=np.float32)
    return y


def _is_canonical_causal(attn_mask):
    m = np.asarray(attn_mask).reshape(S, S)
    iu = np.triu_indices(S, k=1)
    if not np.all(m[iu] <= -1e8):
        return False
    il = np.tril_indices(S, k=0)
    return np.all(m[il] == 0.0)


def _scores_safe(x, Wq, Wk):
    """Sampled bound on |scores| to make exp-without-max safe."""
    rng = np.random.default_rng(0)
    qi = rng.choice(S, 96, replace=False)
    ki = rng.choice(S, 384, replace=False)
    mx = 0.0
    for b in range(B):
        q = (x[b][qi] @ Wq.T) / math.sqrt(d)  # [96, D]
        k = x[b][ki] @ Wk.T  # [384, D]
        qh = q.reshape(96, H, d)
        kh = k.reshape(384, H, d)
        s = np.einsum("qhd,khd->hqk", qh, kh)
        mx = max(mx, float(np.abs(s).max()))
    return mx < 30.0


def _numpy_reference(x, attn_mask, Wq, Wk, Wv, Wo):
    out = np.zeros((B, S, D), dtype=np.float32)
    m = np.asarray(attn_mask, dtype=np.float32).reshape(S, S)
    for b in range(B):
        q = (x[b] @ Wq.T).reshape(S, H, d).transpose(1, 0, 2)
        k = (x[b] @ Wk.T).reshape(S, H, d).transpose(1, 0, 2)
        v = (x[b] @ Wv.T).reshape(S, H, d).transpose(1, 0, 2)
        q = q / np.float32(math.sqrt(d))
        att_out = np.zeros((H, S, d), dtype=np.float32)
        for h in range(H):
            s = q[h] @ k[h].T + m
            s = s - s.max(axis=-1, keepdims=True)
            p = np.exp(s)
            p /= p.sum(axis=-1, keepdims=True)
            att_out[h] = p @ v[h]
        out[b] = att_out.transpose(1, 0, 2).reshape(S, D) @ Wo.T
    return out


def kernel(x, attn_mask, Wq, Wk, Wv, Wo):
    x = np.asarray(x, dtype=np.float32)
    Wq = np.asarray(Wq, dtype=np.float32)
    Wk = np.asarray(Wk, dtype=np.float32)
    Wv = np.asarray(Wv, dtype=np.float32)
    Wo = np.asarray(Wo, dtype=np.float32)

    if not _is_canonical_causal(attn_mask) or not _scores_safe(x, Wq, Wk):
        return _numpy_reference(x, attn_mask, Wq, Wk, Wv, Wo)

    from concourse.bass_utils import run_bass_kernel_spmd

    if "nc" not in _CACHE:
        _CACHE["nc"] = _build_module()
    nc = _CACHE["nc"]

    in_maps = make_in_maps(x, Wq, Wk, Wv, Wo)
    res = run_bass_kernel_spmd(nc, in_maps, core_ids=list(range(N_CORES)))
    return combine_results(res.results)

